# revision 1
# baseline (speedup 1.0000x reference)
"""Trainium2 Bass kernel for nn_AngularSymmetry (B=16, M=64, L=6), 8-core data parallel.

Math (per batch b, output row i, summed over j,k in [0,64)):
  G = coords @ coords.T                      (Gram)
  num[i,j,k]  = nsq[i] - G[i,j] - G[i,k] + G[j,k]     (= vec_ij . vec_ik)
  denp[i,j,k] = (sq2pi*d[i,j])*(sq2pi*d[i,k]) + 2pi*1e-5   (= 2pi*(R_ij R_ik + 1e-5))
  phase = num/denp  (= theta/2pi);  c = cos(2pi*phase) via shifted round-to-int
  and ACT Sin (args kept in (-pi, pi])
  E[i,j,k] = s[i,j]*s[i,k]*s[j,k],  s = exp(-4 d^2)*d_cutoff
  res[i,l] = 2^(1-zeta_l) * sum_jk (1 + lambda_l*c)^zeta_l * E
with (lambda, zeta) = (+1,2),(+1,4),(+1,8),(-1,2),(-1,4),(-1,8).

All (j,k) maps are symmetric in (j,k), so only block families delta =
kblk - jblk >= 0 are computed (16x16 blocks); delta > 0 sums get weight 2
(folded into the reduce scale). num and the Q(=s-matrix) replication are
produced on the TensorEngine via indicator-matmul views of an identity
matrix, accumulated in PSUM.

Each of the 8 cores handles 2 batches (128 partitions = 2*64 (b,i) rows).
"""
import sys

sys.path.insert(0, "/opt/trn_rl_repo")
import contextlib

import numpy as np

import concourse.bass as bass
import concourse.tile as tile
from concourse import bacc, mybir
from concourse.bass_utils import run_bass_kernel_spmd

F32 = mybir.dt.float32
BF16 = mybir.dt.bfloat16
Alu = mybir.AluOpType
Act = mybir.ActivationFunctionType

B, M, L = 16, 64, 6
NCORES = 8
BPC = B // NCORES  # batches per core = 2
P = BPC * M  # 128 partitions
TWO_PI = float(2.0 * np.pi)
SQ2PI = float(np.sqrt(2.0 * np.pi))
MAGIC = 12582912.0  # 1.5 * 2^23 -> fp32 round-to-int via add/sub
EPS2PI = float(2.0 * np.pi * 1e-5)

TB = 16  # triangle block size
NT = M // TB  # 4 block rows/cols
FAMS = [(d, NT - d, 1.0 if d == 0 else 2.0) for d in range(NT)]  # (delta, nblocks, weight)
MAXF = NT * TB * TB  # largest family free size (1024)
SCALES = [1.0 / 2.0, 1.0 / 8.0, 1.0 / 128.0]  # 2^(1-zeta)

_NC = None


def _build(reps=1):
    nc = bacc.Bacc("TRN2", target_bir_lowering=False, debug=False, num_devices=NCORES)
    dcut = nc.dram_tensor("d_cutoff", [BPC, M, M], F32, kind="ExternalInput").ap()
    dd = nc.dram_tensor("d", [BPC, M, M], F32, kind="ExternalInput").ap()
    co = nc.dram_tensor("atom_coordinates", [BPC, M, 3], F32, kind="ExternalInput").ap()
    out = nc.dram_tensor("out", [BPC, M, L], F32, kind="ExternalOutput").ap()
    g_dram = nc.dram_tensor("g_scratch", [BPC, M, M], F32, kind="Internal").ap()
    q_dram = nc.dram_tensor("q_scratch", [BPC, M, M], BF16, kind="Internal").ap()
    q2_dram = nc.dram_tensor("q2_scratch", [BPC, M, M], BF16, kind="Internal").ap()

    with tile.TileContext(nc) as tc:
        with contextlib.ExitStack() as ctx:
            pool = ctx.enter_context(tc.tile_pool(name="w", bufs=1))
            pool2 = ctx.enter_context(tc.tile_pool(name="w2", bufs=3))
            pool4 = ctx.enter_context(tc.tile_pool(name="w4", bufs=4))
            poolB = ctx.enter_context(tc.tile_pool(name="wB", bufs=2))
            poolS = ctx.enter_context(tc.tile_pool(name="wS", bufs=1))
            psp = ctx.enter_context(tc.tile_pool(name="ps", bufs=4, space="PSUM"))

            def _body():
                # ---------- prep ----------
                d_t = pool.tile([P, M], F32, tag="d_t")
                dc_t = pool.tile([P, M], F32, tag="dc_t")
                co_t = pool.tile([P, 3], F32, tag="co_t")
                nc.sync.dma_start(d_t[:], dd.rearrange("b i j -> (b i) j"))
                nc.sync.dma_start(dc_t[:], dcut.rearrange("b i j -> (b i) j"))
                nc.sync.dma_start(co_t[:], co.rearrange("b i d -> (b i) d"))

                # identity (needed for PE transposes)
                ones_t = pool.tile([P, P], F32, tag="ones_t")
                nc.vector.memset(ones_t[:], 1.0)
                idn = pool.tile([P, P], F32, tag="idn")
                nc.gpsimd.affine_select(
                    idn[:], ones_t[:], pattern=[[1, P]], compare_op=Alu.is_equal,
                    fill=0.0, channel_multiplier=-1,
                )

                ct3 = pool.tile([3, P], F32, tag="ct3")
                nc.sync.dma_start(ct3[:], co.rearrange("b i d -> d (b i)"))

                # cross-batch Gram [128,128]: out[p,q] = coords_p . coords_q
                gram_ps = psp.tile([P, P], F32, tag="ps")
                nc.tensor.matmul(gram_ps[:], ct3[:], ct3[:], start=True, stop=True)
                g_sb = pool.tile([P, M], F32, tag="g_sb")
                nc.scalar.copy(g_sb[0:M, :], gram_ps[0:M, 0:M])
                nc.scalar.copy(g_sb[M:P, :], gram_ps[M:P, M:P])

                # nsq_row[0, p] = |coords_p|^2
                sq3t = pool.tile([3, P], F32, tag="sq3t")
                nc.scalar.square(sq3t[:], ct3[:])
                ones3 = pool.tile([3, 1], F32, tag="ones3")
                nc.vector.memset(ones3[:], 1.0)
                nsq_ps = psp.tile([1, P], F32, tag="ps")
                nc.tensor.matmul(nsq_ps[:], ones3[:], sq3t[:], start=True, stop=True)
                nsq_row = pool.tile([1, P], F32, tag="nsq_row")
                nc.scalar.copy(nsq_row[:], nsq_ps[:])

                # ut = sqrt(2pi)*d ; s = exp(-4 d^2)*d_cutoff
                ut = pool.tile([P, M], F32, tag="ut")
                nc.vector.tensor_scalar(ut[:], d_t[:], SQ2PI, None, op0=Alu.mult)
                d2 = pool.tile([P, M], F32, tag="d2")
                nc.scalar.square(d2[:], d_t[:])
                e1 = pool.tile([P, M], F32, tag="e1")
                nc.scalar.activation(e1[:], d2[:], Act.Exp, scale=-4.0)
                s_t = pool.tile([P, M], F32, tag="s_t")
                nc.vector.tensor_tensor(s_t[:], e1[:], dc_t[:], op=Alu.mult)
                s_b16 = pool.tile([P, M], BF16, tag="s_b16")
                nc.vector.tensor_copy(s_b16[:], s_t[:])

                # Qsym = s + s^T per batch (d/d_cutoff are not symmetric; the
                # theta/power factors are, so off-diagonal blocks use Q + Q^T)
                sfull = pool.tile([P, P], F32, tag="sfull")
                nc.vector.memset(sfull[:], 0.0)
                nc.vector.tensor_copy(sfull[0:M, 0:M], s_t[0:M, :])
                nc.vector.tensor_copy(sfull[M:P, M:P], s_t[M:P, :])
                sfT_ps = psp.tile([P, P], F32, tag="ps")
                nc.tensor.transpose(sfT_ps[:], sfull[:], idn[:])
                qsym = pool.tile([P, M], BF16, tag="qsym")
                qsf = pool.tile([P, P], F32, tag="qsf")
                nc.vector.tensor_tensor(qsf[:], sfull[:], sfT_ps[:], op=Alu.add)
                nc.vector.tensor_copy(qsym[0:M, :], qsf[0:M, 0:M])
                nc.vector.tensor_copy(qsym[M:P, :], qsf[M:P, M:P])

                # wG[k0, p] = -G[b(p), i(p), k0]  (shared by the -G_ij and -G_ik matmuls)
                wneg = pool.tile([P, M], F32, tag="wneg")
                nc.scalar.activation(wneg[:], g_sb[:], Act.Copy, bias=0.0, scale=-1.0)
                wG_ps = psp.tile([M, P], F32, tag="ps")
                nc.tensor.transpose(wG_ps[:], wneg[:], idn[:])
                wGx = pool.tile([M + 1, P], F32, tag="wGx")
                nc.scalar.copy(wGx[0:M, :], wG_ps[:])
                nc.scalar.copy(wGx[M : M + 1, :], nsq_row[:])

                # batch indicators [2, P]
                ones2 = pool.tile([2, P], F32, tag="ones2")
                nc.vector.memset(ones2[:], 1.0)
                ind2a = pool.tile([2, P], F32, tag="ind2a")
                nc.gpsimd.affine_select(
                    ind2a[:], ones2[:], pattern=[[1, P]], compare_op=Alu.is_ge,
                    fill=0.0, base=0, channel_multiplier=-M,
                )
                ind2 = pool.tile([2, P], F32, tag="ind2")
                nc.gpsimd.affine_select(
                    ind2[:], ind2a[:], pattern=[[-1, P]], compare_op=Alu.is_ge,
                    fill=0.0, base=M - 1, channel_multiplier=M,
                )
                ind2b = pool.tile([2, P], BF16, tag="ind2b")
                nc.vector.tensor_copy(ind2b[:], ind2[:])

                half_pi = pool.tile([P, 1], F32, tag="half_pi")
                nc.vector.memset(half_pi[:], float(np.pi / 2.0))
                acc = pool.tile([P, L * NT], F32, tag="acc")
                nc.vector.memset(acc[:], 0.0)

                # stage per-batch G and s(=Q) matrices to DRAM, read back flat
                nc.sync.dma_start(g_dram.rearrange("b j k -> (b j) k"), g_sb[:])
                nc.sync.dma_start(q_dram.rearrange("b j k -> (b j) k"), s_b16[:])
                nc.sync.dma_start(q2_dram.rearrange("b j k -> (b j) k"), qsym[:])
                g_flat3 = pool.tile([2, M, M], F32, tag="g_flat")
                nc.sync.dma_start(g_flat3[:], g_dram)
                q_flat3 = pool.tile([2, M, M], BF16, tag="q_flat")
                nc.sync.dma_start(q_flat3[:], q_dram)
                q2_flat3 = pool.tile([2, M, M], BF16, tag="q2_flat")
                nc.sync.dma_start(q2_flat3[:], q2_dram)

                sx_t = {}

                def build_sx(delta, nb):
                    # Sx[c, f] = (j(f)==c) + (k(f)==c) for c<64; Sx[64, f] = 1
                    fs = nb * TB * TB
                    sx = pool.tile([M + 1, fs], F32, tag=f"sx{delta}")
                    jv = (
                        idn[0:M, 0 : nb * TB]
                        .rearrange("c (n j) -> c n j", j=TB)
                        .unsqueeze(3)
                        .broadcast_to([M, nb, TB, TB])
                    )
                    kv = (
                        idn[0:M, delta * TB : (delta + nb) * TB]
                        .rearrange("c (n k) -> c n k", k=TB)
                        .unsqueeze(2)
                        .broadcast_to([M, nb, TB, TB])
                    )
                    nc.vector.tensor_tensor(
                        sx[0:M, 0:fs].rearrange("c (n j k) -> c n j k", j=TB, k=TB),
                        jv, kv, op=Alu.add,
                    )
                    nc.vector.memset(sx[M : M + 1, 0:fs], 1.0)
                    sx_t[delta] = sx

                for _d, _nb, _w in FAMS:
                    build_sx(_d, _nb)

                # ---------- main: triangle block families (2-phase pipeline) ----------
                num_sb = {}
                qq_sb = {}
                rec_t = {}

                # Phase A (per family): PE matmuls for num & Q-replica; PSUM->SBUF
                # copies on ACT (frees PSUM slots); den/recip (no G dependency).
                for delta, nb, wgt in FAMS:
                    fs = nb * TB * TB

                    def jbc(t, nb=nb, delta=delta):
                        v = t[:, 0 : nb * TB].rearrange("p (n j) -> p n j", j=TB)
                        return v.unsqueeze(3).broadcast_to([P, nb, TB, TB])

                    def kbc(t, nb=nb, delta=delta):
                        v = t[:, delta * TB : (delta + nb) * TB].rearrange(
                            "p (n k) -> p n k", k=TB
                        )
                        return v.unsqueeze(2).broadcast_to([P, nb, TB, TB])

                    num_ps = psp.tile([P, MAXF], F32, tag="ps")
                    qq_ps = psp.tile([P, MAXF], F32, tag="ps")
                    for n in range(nb):
                        j0 = n * TB
                        k0 = (n + delta) * TB
                        cols = TB * TB
                        outn = num_ps[:, n * cols : (n + 1) * cols]
                        nc.tensor.matmul(
                            outn, wGx[:], sx_t[delta][:, n * cols : (n + 1) * cols],
                            start=True, stop=False,
                        )
                        rhs_g = g_flat3[:, j0 : j0 + TB, k0 : k0 + TB]
                        nc.tensor.matmul(outn, ind2[:], rhs_g, start=False, stop=True)
                        outq = qq_ps[:, n * cols : (n + 1) * cols]
                        qsrc = q_flat3 if delta == 0 else q2_flat3
                        rhs_q = qsrc[:, j0 : j0 + TB, k0 : k0 + TB]
                        nc.tensor.matmul(outq, ind2b[:], rhs_q, start=True, stop=True)

                    nsb = pool4.tile([P, MAXF], F32, tag="num_sb")
                    nc.scalar.copy(nsb[:, 0:fs], num_ps[:, 0:fs])
                    num_sb[delta] = nsb
                    qsb = pool4.tile([P, MAXF], BF16, tag="qq_sb")
                    nc.scalar.copy(qsb[:, 0:fs], qq_ps[:, 0:fs])
                    qq_sb[delta] = qsb

                    def g4(tl, fs=fs, nb=nb):
                        return tl[:, 0:fs].rearrange("p (n j k) -> p n j k", j=TB, k=TB)

                    den = poolB.tile([P, MAXF], F32, tag="den")
                    nc.gpsimd.tensor_tensor(g4(den), jbc(ut), kbc(ut), op=Alu.mult)
                    nc.vector.tensor_scalar(den[:, 0:fs], den[:, 0:fs], EPS2PI, None, op0=Alu.add)
                    rec = pool4.tile([P, MAXF], F32, tag="rec")
                    recs = poolB.tile([P, MAXF], F32, tag="recs")
                    nc.vector.reciprocal_approx_accurate(rec[:, 0:fs], den[:, 0:fs], recs[:, 0:fs])
                    rec_t[delta] = rec

                # Phase B (per family): phase, range reduction, cos, powers, E, sums
                for delta, nb, wgt in FAMS:
                    fs = nb * TB * TB

                    def jbc(t, nb=nb, delta=delta):
                        v = t[:, 0 : nb * TB].rearrange("p (n j) -> p n j", j=TB)
                        return v.unsqueeze(3).broadcast_to([P, nb, TB, TB])

                    def kbc(t, nb=nb, delta=delta):
                        v = t[:, delta * TB : (delta + nb) * TB].rearrange(
                            "p (n k) -> p n k", k=TB
                        )
                        return v.unsqueeze(2).broadcast_to([P, nb, TB, TB])

                    def g4(tl, fs=fs, nb=nb):
                        return tl[:, 0:fs].rearrange("p (n j k) -> p n j k", j=TB, k=TB)

                    ph = pool2.tile([P, MAXF], F32, tag="ph")
                    nc.vector.tensor_tensor(ph[:, 0:fs], num_sb[delta][:, 0:fs], rec_t[delta][:, 0:fs], op=Alu.mult)

                    # n = round(ph - 0.25); frn = n - ph in [-0.75, 0.25]
                    nr = poolB.tile([P, MAXF], F32, tag="nr")
                    nc.vector.tensor_scalar(nr[:, 0:fs], ph[:, 0:fs], -0.25, MAGIC, op0=Alu.add, op1=Alu.add)
                    nc.vector.tensor_scalar(nr[:, 0:fs], nr[:, 0:fs], -MAGIC, None, op0=Alu.add)
                    frn = pool2.tile([P, MAXF], F32, tag="frn")
                    nc.gpsimd.tensor_tensor(frn[:, 0:fs], nr[:, 0:fs], ph[:, 0:fs], op=Alu.subtract)

                    # c = cos(2pi*fr) = sin(2pi*frn + pi/2), arg in (-pi, pi]
                    c = pool2.tile([P, MAXF], BF16, tag="c")
                    nc.scalar.activation(c[:, 0:fs], frn[:, 0:fs], Act.Sin, bias=half_pi[:], scale=TWO_PI)

                    p2 = pool2.tile([P, MAXF], BF16, tag="p2")
                    nc.scalar.activation(p2[:, 0:fs], c[:, 0:fs], Act.Square, bias=1.0, scale=1.0)
                    m2 = pool2.tile([P, MAXF], BF16, tag="m2")
                    nc.scalar.activation(m2[:, 0:fs], c[:, 0:fs], Act.Square, bias=1.0, scale=-1.0)
                    p4 = pool2.tile([P, MAXF], BF16, tag="p4")
                    nc.scalar.square(p4[:, 0:fs], p2[:, 0:fs])
                    m4 = pool2.tile([P, MAXF], BF16, tag="m4")
                    nc.gpsimd.tensor_tensor(m4[:, 0:fs], m2[:, 0:fs], m2[:, 0:fs], op=Alu.mult)

                    # E = s_j * s_k * Q
                    e0 = poolB.tile([P, MAXF], BF16, tag="e0")
                    nc.gpsimd.tensor_tensor(g4(e0), jbc(s_t), kbc(s_t), op=Alu.mult)
                    ee = pool2.tile([P, MAXF], BF16, tag="ee")
                    nc.vector.tensor_tensor(ee[:, 0:fs], e0[:, 0:fs], qq_sb[delta][:, 0:fs], op=Alu.mult)

                    # plus branch: DVE muls + scaled ACT reduces
                    a2p = pool2.tile([P, MAXF], BF16, tag="a2_0")
                    nc.vector.tensor_tensor(a2p[:, 0:fs], p2[:, 0:fs], ee[:, 0:fs], op=Alu.mult)
                    a4p = poolB.tile([P, MAXF], BF16, tag="a4_0")
                    nc.vector.tensor_tensor(a4p[:, 0:fs], p2[:, 0:fs], a2p[:, 0:fs], op=Alu.mult)
                    a8p = poolB.tile([P, MAXF], BF16, tag="a8_0")
                    nc.vector.affine_mul_reduce(
                        a8p[:, 0:fs], acc[:, 2 * NT + delta : 2 * NT + delta + 1],
                        p4[:, 0:fs], a4p[:, 0:fs], 1.0, 0.0,
                    )
                    for idx, a in enumerate([a2p, a4p]):
                        dst = acc[:, idx * NT + delta : idx * NT + delta + 1]
                        red = poolS.tile([P, MAXF], BF16, tag="redsink")
                        nc.scalar.activation(
                            red[:, 0:fs], a[:, 0:fs], Act.Copy, bias=0.0,
                            scale=SCALES[idx], accum_out=dst,
                        )
                    # minus branch: DVE muls into one tile + single 3-way reduce
                    am = poolB.tile([P, 3, MAXF], BF16, tag="am")
                    a2m, a4m, a8m = am[:, 0, :], am[:, 1, :], am[:, 2, :]
                    accv = acc[:].rearrange("p (l c) -> p l c", c=NT)
                    nc.vector.affine_mul_reduce(
                        a2m[:, 0:fs], accv[:, 3, delta : delta + 1], m2[:, 0:fs], ee[:, 0:fs], 1.0, 0.0
                    )
                    nc.vector.affine_mul_reduce(
                        a4m[:, 0:fs], accv[:, 4, delta : delta + 1], m2[:, 0:fs], a2m[:, 0:fs], 1.0, 0.0
                    )
                    nc.vector.affine_mul_reduce(
                        a8m[:, 0:fs], accv[:, 5, delta : delta + 1], m4[:, 0:fs], a4m[:, 0:fs], 1.0, 0.0
                    )

                # ---------- finish: sum families, store ----------
                res = pool.tile([P, L], F32, tag="res")
                nc.vector.tensor_reduce(
                    res[:], acc[:].rearrange("p (l c) -> p l c", c=NT),
                    axis=mybir.AxisListType.X, op=Alu.add,
                )
                nc.vector.tensor_scalar(res[:, 2:3], res[:, 2:3], SCALES[2], None, op0=Alu.mult)
                for idx in range(3):
                    nc.vector.tensor_scalar(
                        res[:, 3 + idx : 4 + idx], res[:, 3 + idx : 4 + idx],
                        SCALES[idx], None, op0=Alu.mult,
                    )
                nc.sync.dma_start(out.rearrange("b i l -> (b i) l"), res[:])

            for _rep in range(reps):
                _body()

    nc.compile()
    return nc


def _get_nc():
    global _NC
    if _NC is None:
        _NC = _build()
    return _NC


_RUNNER = None


def _get_runner():
    """Cached jitted SPMD runner (run_bass_kernel_spmd re-lowers per call;
    this builds the PJRT executable once and reuses it)."""
    global _RUNNER
    if _RUNNER is not None:
        return _RUNNER
    import jax
    from jax.sharding import Mesh, PartitionSpec
    from jax.experimental.shard_map import shard_map
    from concourse import bass2jax
    from concourse.bass2jax import _bass_exec_p, install_neuronx_cc_hook

    nc = _get_nc()
    install_neuronx_cc_hook()
    partition_name = nc.partition_id_tensor.name if nc.partition_id_tensor else None
    in_names, out_names, out_avals, zero_outs = [], [], [], []
    for alloc in nc.m.functions[0].allocations:
        if not isinstance(alloc, mybir.MemoryLocationSet):
            continue
        name = alloc.memorylocations[0].name
        if alloc.kind == "ExternalInput":
            if name != partition_name:
                in_names.append(name)
        elif alloc.kind == "ExternalOutput":
            shape = tuple(alloc.tensor_shape)
            dtype = mybir.dt.np(alloc.dtype)
            out_names.append(name)
            out_avals.append(jax.core.ShapedArray(shape, dtype))
            zero_outs.append(np.zeros(shape, dtype))
    all_names = in_names + out_names + ([partition_name] if partition_name else [])

    def one(*args):
        ops = list(args)
        if partition_name is not None:
            ops.append(bass2jax.partition_id_tensor())
        return tuple(
            _bass_exec_p.bind(
                *ops,
                out_avals=tuple(out_avals),
                in_names=tuple(all_names),
                out_names=tuple(out_names),
                lowering_input_output_aliases=(),
                sim_require_finite=True,
                sim_require_nnan=True,
                nc=nc,
            )
        )

    devices = jax.devices()[:NCORES]
    mesh = Mesh(np.asarray(devices), ("core",))
    specs = (PartitionSpec("core"),) * (len(in_names) + len(out_names))
    out_specs = (PartitionSpec("core"),) * len(out_names)
    fn = jax.jit(
        shard_map(one, mesh=mesh, in_specs=specs, out_specs=out_specs, check_rep=False),
        keep_unused=True,
    )
    concat_zeros = [
        np.zeros((NCORES * z.shape[0], *z.shape[1:]), z.dtype) for z in zero_outs
    ]
    _RUNNER = (fn, in_names, out_names, out_avals, concat_zeros)
    return _RUNNER


def kernel(d_cutoff, d, atom_coordinates):
    full = {
        "d_cutoff": np.ascontiguousarray(d_cutoff, dtype=np.float32),
        "d": np.ascontiguousarray(d, dtype=np.float32),
        "atom_coordinates": np.ascontiguousarray(atom_coordinates, dtype=np.float32),
    }
    fn, in_names, out_names, out_avals, concat_zeros = _get_runner()
    concat_in = [full[name] for name in in_names]  # [B,...] == concat of per-core [BPC,...]
    outs = fn(*concat_in, *concat_zeros)
    oi = out_names.index("out")
    return np.asarray(outs[oi]).reshape(B, M, L)


if __name__ == "__main__":
    rng = np.random.default_rng(0)
    inputs = {
        "d_cutoff": rng.uniform(0, 1, (B, M, M)).astype(np.float32),
        "d": rng.uniform(0, 1, (B, M, M)).astype(np.float32),
        "atom_coordinates": rng.standard_normal((B, M, 3)).astype(np.float32),
    }
    out = kernel(**inputs)
    print("kernel out shape:", out.shape, "sample:", out[0, 0])

